# revision 8
# baseline (speedup 1.0000x reference)
"""Low-rank ray tracer CSI kernel for 8 Trainium2 NeuronCores.

v5: fp8 error-feedback stream + TensorE DoubleRow p-fold, contiguous APs.

    csi[k] = (1/D) * f_k^T (Ua^T Ub) f_k,   Ua[d,r] = sum_p ua[d,p,r]

The kernel is HBM-stream-bound (~310-358 GB/s/core), so the main lever is
bytes: ua/ub ship as fp8 e4m3 (4.2 MB/core vs 8.4 fp16).  Plain e4m3 rounding
fails the 2e-2 gate (2.9e-2 end-to-end); the host instead quantizes with
error feedback along p (q[p] = e4m3(x[p] + carry)), which telescopes the
p-sum error to the final carry — 1.7e-3 end-to-end on the harness inputs.

PE can't matmul int8, but fp8e4 DoubleRow contracts 2 k-tiles per pass: with
a doubled identity [128, 2, 128] stationary, out = X[:,0,:] + X[:,1,:] — a
pairwise p-add at 2x fp16 throughput (HW-validated exact).  Host layout
[D, 4, 2, R, 32] (p = j*64 + t*32 + f) makes each matmul's moving AP a run
of rc*32 contiguous bytes per k-tile.  Each [DC, ., rc, .] r-chunk folds
256 -> 32 via 4 accumulating DR matmuls (PSUM fp32); DVE reduces the
32-wide tail.  A burst of dummy DR matmuls at t~6us walks the PE out of its
low-power pstate before the first real fold.

Chunk DMAs taper (4,8,16,16,12,8 r's: small first so the fold pipeline
starts ~9us, small last so the drain is short) and alternate between the two
HWDGE rings (sync/scalar).  No nc.scalar compute ops -> no ACT_TABLE_LOAD.
"""

import sys

import ml_dtypes
import numpy as np

sys.path.insert(0, "/opt/trn_rl_repo")

import concourse.bacc as bacc
import concourse.bass as bass
import concourse.mybir as mybir
from concourse.bass_utils import run_bass_kernel_spmd
from concourse.masks import make_identity

from concourse.tile import TileContext

D, P, R, K = 1024, 256, 64, 1024
NCORES = 8
DC = D // NCORES  # directions per core
RCS = (4, 8, 16, 16, 12, 8)  # r-chunk taper
KC = K // 128  # k chunks of 128 (PSUM partition limit)
PF = 32  # p-fold tail width: 256 -> 32 via 4 DoubleRow matmuls per chunk
NWARM = 10  # dummy DR matmuls to ramp the PE pstate before the first fold

F32 = mybir.dt.float32
F16 = mybir.dt.float16
F8 = mybir.dt.float8e4
E4M3 = ml_dtypes.float8_e4m3


def build_bass() -> bass.Bass:
    nc = bacc.Bacc(None, target_bir_lowering=False)
    # per-core shards, host layout [d, j, t, r, f] with p = j*64 + t*32 + f,
    # fp8 e4m3 (error-feedback quantized along p on host)
    ua = nc.declare_dram_parameter("ua", [DC, 4, 2, R, PF], F8, isOutput=False)
    ub = nc.declare_dram_parameter("ub", [DC, 4, 2, R, PF], F8, isOutput=False)
    # F^T only; the [p, c, r] layout (k = c*128 + p) is rebuilt on-device
    ft = nc.declare_dram_parameter("ft", [R, K], F16, isOutput=False)
    # out[p, c] = partial csi[c*128 + p], already scaled by 1/D
    out = nc.declare_dram_parameter("out", [128, KC], F32, isOutput=True)

    with TileContext(nc) as tc:
        with (
            tc.tile_pool(name="const", bufs=1) as cpool,
            tc.tile_pool(name="chunks", bufs=12) as chpool,
            tc.tile_pool(name="small", bufs=1) as spool,
            tc.tile_pool(name="scratch", bufs=1) as scpool,
            tc.tile_pool(name="pwarm", bufs=1, space="PSUM") as wpool,
            tc.tile_pool(name="pfold", bufs=2, space="PSUM") as fpool,
            tc.tile_pool(name="ptp", bufs=2, space="PSUM") as tppool,
            tc.tile_pool(name="pfinal", bufs=1, space="PSUM") as ppool1,
        ):
            # doubled identity [128, 2, 128] fp8: dident[i, t, m] = (i == m);
            # DoubleRow with it as stationary computes X[:,0,:] + X[:,1,:]
            dident = cpool.tile([128, 2, 128], F8)
            nc.gpsimd.memset(dident[:], 0.0)
            nc.gpsimd.affine_select(
                out=dident[:],
                in_=dident[:],
                compare_op=mybir.AluOpType.not_equal,
                fill=1.0,
                base=0,
                pattern=[[0, 2], [-1, 128]],
                channel_multiplier=1,
            )
            ident = cpool.tile([128, 128], F16)
            make_identity(nc, ident[:])

            # PE pstate warmup: dummy DR matmuls on dident while the first
            # chunks are still in flight (PE needs ~3us busy to leave the
            # low-power state)
            warm = wpool.tile([128, 128], F32)
            for _ in range(NWARM):
                nc.tensor.matmul(
                    warm[:],
                    dident[:],
                    dident[:],
                    start=True,
                    stop=True,
                    perf_mode=mybir.MatmulPerfMode.DoubleRow,
                )

            ft_sb = cpool.tile([R, K], F16)

            # Streaming p-reduction: Ua[d,r] = sum_p ua[d,r,p] (same for ub).
            # ua chunks ride the sync HWDGE ring, ub chunks the scalar ring;
            # ft slots in third on the scalar ring so the mid-stream
            # transposes never stall PE.
            u_a = spool.tile([DC, R], F16, tag="u_a")
            u_b = spool.tile([DC, R], F16, tag="u_b")
            fpc_sb = cpool.tile([128, KC, R], F16)

            def fold_chunk(ch, u, r0, rc):
                fold = fpool.tile([DC, rc, PF], F32, tag="fold")
                for j in range(4):
                    nc.tensor.matmul(
                        fold[:].rearrange("q r f -> q (r f)"),
                        dident[:],
                        ch[:, j],
                        start=(j == 0),
                        stop=(j == 3),
                        perf_mode=mybir.MatmulPerfMode.DoubleRow,
                    )
                nc.vector.tensor_reduce(
                    out=u[:, r0 : r0 + rc],
                    in_=fold[:],
                    axis=mybir.AxisListType.X,
                    op=mybir.AluOpType.add,
                )

            r0 = 0
            with nc.allow_low_precision(reason="p-sum of 256 fp8 EF-quantized"):
                for ci, rc in enumerate(RCS):
                    for t_ap, u, ring in ((ua, u_a, nc.sync), (ub, u_b, nc.scalar)):
                        ch = chpool.tile([DC, 4, 2, rc, PF], F8, tag="chunk")
                        ring.dma_start(out=ch[:], in_=t_ap[:, :, :, r0 : r0 + rc, :])
                        if ci == 2 and u is u_b:
                            # PE is the tighter engine mid-stream: DVE
                            # direct-reduces this chunk straight from SBUF
                            nc.vector.tensor_reduce(
                                out=u[:, r0 : r0 + rc],
                                in_=ch[:].rearrange("q j t r f -> q r j t f"),
                                axis=mybir.AxisListType.XYZ,
                                op=mybir.AluOpType.add,
                            )
                        else:
                            fold_chunk(ch, u, r0, rc)
                    r0 += rc
                    if ci == 1:
                        nc.scalar.dma_start(out=ft_sb[:], in_=ft[:])
                    if ci == 3:
                        # rebuild F as [p, c, r] blocks from F^T via PE
                        # transposes; scheduled here so they fill the PE gap
                        # while the last big chunks are still streaming
                        for c in range(KC):
                            tp = tppool.tile([128, R], F16, tag="tp")
                            nc.tensor.transpose(
                                tp[:], ft_sb[:, c * 128 : (c + 1) * 128], ident[0:R, 0:R]
                            )
                            nc.vector.tensor_copy(out=fpc_sb[:, c, :], in_=tp[:])

            # Gram matrix M[r1,r2] = sum_d Ua[d,r1] Ub[d,r2]
            m_psum = ppool1.tile([R, R], F32, tag="m")
            nc.tensor.matmul(m_psum[:], u_a[:], u_b[:], start=True, stop=True)
            # fold the 1/D normalization into M while copying out of PSUM
            m_sb = spool.tile([R, R], F16)
            nc.vector.tensor_scalar_mul(m_sb[:], m_psum[:], 1.0 / D)

            # csi[k] = sum_r2 (sum_r1 F[k,r1] (M/D)[r1,r2]) * F[k,r2]
            # All KC g-matmuls land in ONE psum bank [128, KC*R*4B = 2KB], so
            # the tail is 8 tiny matmuls + one DVE mul + one DVE reduce.
            g_all = ppool1.tile([128, KC, R], F32, tag="g_all")
            for c in range(KC):
                nc.tensor.matmul(
                    g_all[:, c, :],
                    ft_sb[:, c * 128 : (c + 1) * 128],
                    m_sb[:],
                    start=True,
                    stop=True,
                )
            scr = scpool.tile([128, KC, R], F16, tag="scr")
            nc.vector.tensor_mul(out=scr[:], in0=g_all[:], in1=fpc_sb[:])
            csi = spool.tile([128, KC], F32)
            with nc.allow_low_precision(reason="fp16 products, |csi|<2e3, gate 2e-2"):
                nc.vector.tensor_reduce(
                    out=csi[:],
                    in_=scr[:],
                    axis=mybir.AxisListType.X,
                    op=mybir.AluOpType.add,
                )
            nc.scalar.dma_start(out=out[:], in_=csi[:], single_packet=True)
    nc.compile()
    return nc


_NC_CACHE = None


EXTRA_INPUT_NAMES = ("ft",)


def _ef_quant_t(x: np.ndarray) -> np.ndarray:
    """Error-feedback e4m3 quantization along p of [D, P, R] input; returns
    a [D, 4, 2, R, 32] fp8 array (p = j*64 + t*32 + f).  sum_p q[d,p,r] =
    sum_p x[d,p,r] - final_carry[d,r], so the p-sum error is one rounding
    step, not sqrt(P) accumulated noise."""
    q = np.empty((x.shape[0], x.shape[2], x.shape[1]), dtype=E4M3)  # [D, R, P]
    carry = np.zeros((x.shape[0], x.shape[2]), dtype=np.float32)
    for p in range(x.shape[1]):
        v = x[:, p, :] + carry
        qv = v.astype(E4M3)
        carry = v - qv.astype(np.float32)
        q[:, :, p] = qv
    # [D, R, P] -> [D, R, 4, 2, 32] -> [D, 4, 2, R, 32]
    q = q.reshape(x.shape[0], x.shape[2], 4, 2, PF).transpose(0, 2, 3, 1, 4)
    return np.ascontiguousarray(q)


def _prep(inputs):
    ua = np.asarray(inputs["attenuation_vectors"], dtype=np.float32)
    ub = np.asarray(inputs["radiation_vectors"], dtype=np.float32)
    f = np.ascontiguousarray(inputs["frequency_basis_vectors"], dtype=np.float32)

    ua_t = _ef_quant_t(ua)
    ub_t = _ef_quant_t(ub)
    ft = np.ascontiguousarray(f.T.astype(np.float16))
    return ua_t, ub_t, ft


def kernel(**inputs: np.ndarray) -> np.ndarray:
    global _NC_CACHE
    ua_t, ub_t, ft = _prep(inputs)

    if _NC_CACHE is None:
        _NC_CACHE = build_bass()
    nc = _NC_CACHE

    in_maps = [
        {
            "ua": ua_t[c * DC : (c + 1) * DC],
            "ub": ub_t[c * DC : (c + 1) * DC],
            "ft": ft,
        }
        for c in range(NCORES)
    ]
    res = run_bass_kernel_spmd(nc, in_maps, list(range(NCORES)))
    acc = np.zeros((128, KC), dtype=np.float32)
    for r in res.results:
        acc += r["out"]
    return acc.T.reshape(K).astype(np.float32)


if __name__ == "__main__":
    rng = np.random.default_rng(0)
    ins = {
        "attenuation_vectors": rng.standard_normal((D, P, R), dtype=np.float32),
        "radiation_vectors": rng.standard_normal((D, P, R), dtype=np.float32),
        "frequency_basis_vectors": rng.standard_normal((K, R), dtype=np.float32),
    }
    got = kernel(**ins)
    ua_s = ins["attenuation_vectors"].sum(axis=1)
    ub_s = ins["radiation_vectors"].sum(axis=1)
    a = ua_s @ ins["frequency_basis_vectors"].T
    b = ub_s @ ins["frequency_basis_vectors"].T
    want = (a * b).sum(axis=0) / D
    err = np.abs(got - want).max() / np.abs(want).max()
    print("rel err vs local numpy:", err)


# revision 11
# speedup vs baseline: 1.0543x; 1.0543x over previous
"""Low-rank ray tracer CSI kernel for 8 Trainium2 NeuronCores.

v5: fp8 error-feedback stream + TensorE DoubleRow p-fold, contiguous APs.

    csi[k] = (1/D) * f_k^T (Ua^T Ub) f_k,   Ua[d,r] = sum_p ua[d,p,r]

The kernel is HBM-stream-bound (~310-358 GB/s/core), so the main lever is
bytes: ua/ub ship as fp8 e4m3 (4.2 MB/core vs 8.4 fp16).  Plain e4m3 rounding
fails the 2e-2 gate (2.9e-2 end-to-end); the host instead quantizes with
error feedback along p (q[p] = e4m3(x[p] + carry)), which telescopes the
p-sum error to the final carry — 1.7e-3 end-to-end on the harness inputs.

PE can't matmul int8, but fp8e4 DoubleRow contracts 2 k-tiles per pass: with
a doubled identity [128, 2, 128] stationary, out = X[:,0,:] + X[:,1,:] — a
pairwise p-add at 2x fp16 throughput (HW-validated exact).  Host layout
[D, 4, 2, R, 32] (p = j*64 + t*32 + f) makes each matmul's moving AP a run
of rc*32 contiguous bytes per k-tile.  Each [DC, ., rc, .] r-chunk folds
256 -> 32 via 4 accumulating DR matmuls (PSUM fp32); DVE reduces the
32-wide tail.  A burst of dummy DR matmuls at t~6us walks the PE out of its
low-power pstate before the first real fold.

Chunk DMAs taper (4,8,16,16,12,8 r's: small first so the fold pipeline
starts ~9us, small last so the drain is short) and alternate between the two
HWDGE rings (sync/scalar).  No nc.scalar compute ops -> no ACT_TABLE_LOAD.
"""

import sys

import ml_dtypes
import numpy as np

sys.path.insert(0, "/opt/trn_rl_repo")

import concourse.bacc as bacc
import concourse.bass as bass
import concourse.mybir as mybir
from concourse.bass_utils import run_bass_kernel_spmd
from concourse.masks import make_identity

from concourse.tile import TileContext

D, P, R, K = 1024, 256, 64, 1024
NCORES = 8
DC = D // NCORES  # directions per core
RCS = (4, 8, 16, 16, 12, 8)  # r-chunk taper
KC = K // 128  # k chunks of 128 (PSUM partition limit)
PF = 32  # p-fold tail width: 256 -> 32 via 4 DoubleRow matmuls per chunk
NWARM = 10  # dummy DR matmuls to ramp the PE pstate before the first fold

F32 = mybir.dt.float32
F16 = mybir.dt.float16
F8 = mybir.dt.float8e4
E4M3 = ml_dtypes.float8_e4m3


def build_bass() -> bass.Bass:
    nc = bacc.Bacc(None, target_bir_lowering=False)
    # per-core shards, host layout [d, j, t, r, f] with p = j*64 + t*32 + f,
    # fp8 e4m3 (error-feedback quantized along p on host)
    ua = nc.declare_dram_parameter("ua", [DC, 4, 2, R, PF], F8, isOutput=False)
    ub = nc.declare_dram_parameter("ub", [DC, 4, 2, R, PF], F8, isOutput=False)
    # F^T only; the [p, c, r] layout (k = c*128 + p) is rebuilt on-device
    ft = nc.declare_dram_parameter("ft", [R, K], F16, isOutput=False)
    # out[p, c] = partial csi[c*128 + p], already scaled by 1/D
    out = nc.declare_dram_parameter("out", [128, KC], F32, isOutput=True)

    with TileContext(nc) as tc:
        with (
            tc.tile_pool(name="const", bufs=1) as cpool,
            tc.tile_pool(name="chunks", bufs=12) as chpool,
            tc.tile_pool(name="small", bufs=1) as spool,
            tc.tile_pool(name="scratch", bufs=1) as scpool,
            tc.tile_pool(name="pwarm", bufs=1, space="PSUM") as wpool,
            tc.tile_pool(name="pfold", bufs=3, space="PSUM") as fpool,
            tc.tile_pool(name="ptp", bufs=2, space="PSUM") as tppool,
            tc.tile_pool(name="pfinal", bufs=1, space="PSUM") as ppool1,
        ):
            # doubled identity [128, 2, 128] fp8: dident[i, t, m] = (i == m);
            # DoubleRow with it as stationary computes X[:,0,:] + X[:,1,:]
            dident = cpool.tile([128, 2, 128], F8)
            nc.gpsimd.memset(dident[:], 0.0)
            nc.gpsimd.affine_select(
                out=dident[:],
                in_=dident[:],
                compare_op=mybir.AluOpType.not_equal,
                fill=1.0,
                base=0,
                pattern=[[0, 2], [-1, 128]],
                channel_multiplier=1,
            )
            ident = cpool.tile([128, 128], F16)
            make_identity(nc, ident[:])

            # PE pstate warmup: dummy DR matmuls on dident while the first
            # chunks are still in flight (PE needs ~3us busy to leave the
            # low-power state)
            warm = wpool.tile([128, 128], F32)
            for _ in range(NWARM):
                nc.tensor.matmul(
                    warm[:],
                    dident[:],
                    dident[:],
                    start=True,
                    stop=True,
                    perf_mode=mybir.MatmulPerfMode.DoubleRow,
                )

            ft_sb = cpool.tile([R, K], F16)

            # Streaming p-reduction: Ua[d,r] = sum_p ua[d,r,p] (same for ub).
            # ua chunks ride the sync HWDGE ring, ub chunks the scalar ring;
            # ft slots in third on the scalar ring so the mid-stream
            # transposes never stall PE.
            u_a = spool.tile([DC, R], F16, tag="u_a")
            u_b = spool.tile([DC, R], F16, tag="u_b")
            fpc_sb = cpool.tile([128, KC, R], F16)

            def fold_chunk(ch, u, r0, rc):
                fold = fpool.tile([DC, rc, PF], F32, tag="fold")
                for j in range(4):
                    nc.tensor.matmul(
                        fold[:].rearrange("q r f -> q (r f)"),
                        dident[:],
                        ch[:, j],
                        start=(j == 0),
                        stop=(j == 3),
                        perf_mode=mybir.MatmulPerfMode.DoubleRow,
                    )
                nc.vector.tensor_reduce(
                    out=u[:, r0 : r0 + rc],
                    in_=fold[:],
                    axis=mybir.AxisListType.X,
                    op=mybir.AluOpType.add,
                )

            r0 = 0
            with nc.allow_low_precision(reason="p-sum of 256 fp8 EF-quantized"):
                for ci, rc in enumerate(RCS):
                    for t_ap, u in ((ua, u_a), (ub, u_b)):
                        ch = chpool.tile([DC, 4, 2, rc, PF], F8, tag="chunk")
                        nc.sync.dma_start(out=ch[:], in_=t_ap[:, :, :, r0 : r0 + rc, :])
                        fold_chunk(ch, u, r0, rc)
                    r0 += rc
                    if ci == 0:
                        nc.scalar.dma_start(out=ft_sb[:], in_=ft[:])
                    if ci == 3:
                        # rebuild F as [p, c, r] blocks from F^T via PE
                        # transposes; scheduled here so they fill the PE gap
                        # while the last big chunks are still streaming
                        for c in range(KC):
                            tp = tppool.tile([128, R], F16, tag="tp")
                            nc.tensor.transpose(
                                tp[:], ft_sb[:, c * 128 : (c + 1) * 128], ident[0:R, 0:R]
                            )
                            nc.vector.tensor_copy(out=fpc_sb[:, c, :], in_=tp[:])

            # Gram matrix M[r1,r2] = sum_d Ua[d,r1] Ub[d,r2]
            m_psum = ppool1.tile([R, R], F32, tag="m")
            nc.tensor.matmul(m_psum[:], u_a[:], u_b[:], start=True, stop=True)
            # fold the 1/D normalization into M while copying out of PSUM
            m_sb = spool.tile([R, R], F16)
            nc.vector.tensor_scalar_mul(m_sb[:], m_psum[:], 1.0 / D)

            # csi[k] = sum_r2 (sum_r1 F[k,r1] (M/D)[r1,r2]) * F[k,r2]
            # All KC g-matmuls land in ONE psum bank [128, KC*R*4B = 2KB], so
            # the tail is 8 tiny matmuls + one DVE mul + one DVE reduce.
            g_all = ppool1.tile([128, KC, R], F32, tag="g_all")
            for c in range(KC):
                nc.tensor.matmul(
                    g_all[:, c, :],
                    ft_sb[:, c * 128 : (c + 1) * 128],
                    m_sb[:],
                    start=True,
                    stop=True,
                )
            scr = scpool.tile([128, KC, R], F16, tag="scr")
            nc.vector.tensor_mul(out=scr[:], in0=g_all[:], in1=fpc_sb[:])
            csi = spool.tile([128, KC], F32)
            with nc.allow_low_precision(reason="fp16 products, |csi|<2e3, gate 2e-2"):
                nc.vector.tensor_reduce(
                    out=csi[:],
                    in_=scr[:],
                    axis=mybir.AxisListType.X,
                    op=mybir.AluOpType.add,
                )
            nc.scalar.dma_start(out=out[:], in_=csi[:])
    nc.compile()
    return nc


_NC_CACHE = None


EXTRA_INPUT_NAMES = ("ft",)


def _ef_quant_t(x: np.ndarray) -> np.ndarray:
    """Error-feedback e4m3 quantization along p of [D, P, R] input; returns
    a [D, 4, 2, R, 32] fp8 array (p = j*64 + t*32 + f).  sum_p q[d,p,r] =
    sum_p x[d,p,r] - final_carry[d,r], so the p-sum error is one rounding
    step, not sqrt(P) accumulated noise."""
    q = np.empty((x.shape[0], x.shape[2], x.shape[1]), dtype=E4M3)  # [D, R, P]
    carry = np.zeros((x.shape[0], x.shape[2]), dtype=np.float32)
    for p in range(x.shape[1]):
        v = x[:, p, :] + carry
        qv = v.astype(E4M3)
        carry = v - qv.astype(np.float32)
        q[:, :, p] = qv
    # [D, R, P] -> [D, R, 4, 2, 32] -> [D, 4, 2, R, 32]
    q = q.reshape(x.shape[0], x.shape[2], 4, 2, PF).transpose(0, 2, 3, 1, 4)
    return np.ascontiguousarray(q)


def _prep(inputs):
    ua = np.asarray(inputs["attenuation_vectors"], dtype=np.float32)
    ub = np.asarray(inputs["radiation_vectors"], dtype=np.float32)
    f = np.ascontiguousarray(inputs["frequency_basis_vectors"], dtype=np.float32)

    ua_t = _ef_quant_t(ua)
    ub_t = _ef_quant_t(ub)
    ft = np.ascontiguousarray(f.T.astype(np.float16))
    return ua_t, ub_t, ft


def kernel(**inputs: np.ndarray) -> np.ndarray:
    global _NC_CACHE
    ua_t, ub_t, ft = _prep(inputs)

    if _NC_CACHE is None:
        _NC_CACHE = build_bass()
    nc = _NC_CACHE

    in_maps = [
        {
            "ua": ua_t[c * DC : (c + 1) * DC],
            "ub": ub_t[c * DC : (c + 1) * DC],
            "ft": ft,
        }
        for c in range(NCORES)
    ]
    res = run_bass_kernel_spmd(nc, in_maps, list(range(NCORES)))
    acc = np.zeros((128, KC), dtype=np.float32)
    for r in res.results:
        acc += r["out"]
    return acc.T.reshape(K).astype(np.float32)


if __name__ == "__main__":
    rng = np.random.default_rng(0)
    ins = {
        "attenuation_vectors": rng.standard_normal((D, P, R), dtype=np.float32),
        "radiation_vectors": rng.standard_normal((D, P, R), dtype=np.float32),
        "frequency_basis_vectors": rng.standard_normal((K, R), dtype=np.float32),
    }
    got = kernel(**ins)
    ua_s = ins["attenuation_vectors"].sum(axis=1)
    ub_s = ins["radiation_vectors"].sum(axis=1)
    a = ua_s @ ins["frequency_basis_vectors"].T
    b = ub_s @ ins["frequency_basis_vectors"].T
    want = (a * b).sum(axis=0) / D
    err = np.abs(got - want).max() / np.abs(want).max()
    print("rel err vs local numpy:", err)
